# revision 6
# baseline (speedup 1.0000x reference)
"""Trainium2 Bass kernel for a 3-layer ResGatedGraphConv GNN (ClinicalGatedGCN).

Strategy (8 NeuronCores, SPMD):
  - Nodes are partitioned into 8 contiguous ranges (rank-blocked ids, padded to
    a multiple of 128 per rank). Edges are assigned to the rank that owns their
    dst node, grouped by 128-wide dst blocks, and sorted by (epoch, dst) on the
    host (epoch = whether the src row id fits the int16 gather index range).
  - Each rank computes the full [q|v] node table (node-major, one matmul per
    128-node block with the h chunk as the stationary operand, N=256) into a
    single local HBM table, plus a rank-local k table indexed by dst. Per edge
    group one dma_gather per epoch fetches [q|v] src rows into one tile, and a
    single merged gather fetches k[dst] for both epochs.
  - Gate math runs once per group over both epochs' slots in packed layouts
    (DVE 2x mode); segment-sum over dst is a PE matmul against a 0/1 selector
    built on-device with is_equal; both epochs accumulate in one PSUM group.
  - h updates (leaky+BN) run on the scalar engine per AllGather chunk; the
    per-layer h AllGather is split into 4 chunks issued inside the edge loop so
    the collective overlaps edge processing, and the next layer's s/k/qv table
    production is interleaved per chunk to hide the node phase.
  - Mean-pool per graph is a matmul against a host-built indicator with 1/cnt
    folded in; partial pools are AllGather'd and summed; the tiny classifier
    runs on every core.
"""

import numpy as np
import ml_dtypes

import concourse.bacc as bacc
import concourse.bass as bass
import concourse.mybir as mybir
import concourse.tile as tile
from concourse.bass_utils import run_bass_kernel_spmd
from concourse.masks import make_identity

F32 = mybir.dt.float32
BF16 = mybir.dt.bfloat16
I16 = mybir.dt.int16
AF = mybir.ActivationFunctionType
OP = mybir.AluOpType

# ---------------- problem constants (hardcoded per spec) ----------------
N, E, H, G, NCLIN, NCLS = 50000, 800000, 128, 64, 16, 2
NLAYER = 3
EPS = 1e-5
SLOPE = 0.01
R = 8                      # ranks / NeuronCores
SPLIT = 32768              # int16 gather index limit -> 2 epochs

NPR = (N + R - 1) // R     # real nodes per rank
NGRP = (NPR + 127) // 128  # 128-node groups per rank
NPAD = NGRP * 128          # padded nodes per rank
NTOT = R * NPAD            # rank-blocked total rows

CHUNKS = [16, 16, 12, 5]   # edge groups per AllGather chunk
assert sum(CHUNKS) == NGRP


def wrap_idxs_block(idx):
    """Wrap one gather call's indices: idx j -> [j%16, j//16], tiled to 128 parts."""
    n = len(idx)
    assert n % 16 == 0
    w = np.asarray(idx, np.int16).reshape(n // 16, 16).T
    return np.tile(w, (8, 1))


def colmaj128(v):
    """Edge-scalar array -> [128, n/128] layout (edge j at [j%128, j//128])."""
    v = np.asarray(v)
    n = v.shape[0]
    assert n % 128 == 0
    return v.reshape(n // 128, 128).T.copy()


# ---------------------------------------------------------------------------
# host-side preprocessing
# ---------------------------------------------------------------------------

def prep(inputs):
    dtab = ml_dtypes.bfloat16
    x = np.asarray(inputs["x"], np.float32)
    edge_index = np.asarray(inputs["edge_index"])
    edge_attr = np.asarray(inputs["edge_attr"], np.float32)[:, 0]
    batch = np.asarray(inputs["batch"]).astype(np.int64)
    clinical = np.asarray(inputs["clinical"], np.float32)
    Wk, bk = np.asarray(inputs["Wk"], np.float32), np.asarray(inputs["bk"], np.float32)
    Wq, bq = np.asarray(inputs["Wq"], np.float32), np.asarray(inputs["bq"], np.float32)
    Wv, bv = np.asarray(inputs["Wv"], np.float32), np.asarray(inputs["bv"], np.float32)
    Ws, bs = np.asarray(inputs["Ws"], np.float32), np.asarray(inputs["bs"], np.float32)
    We, be = np.asarray(inputs["We"], np.float32), np.asarray(inputs["be"], np.float32)
    gamma = np.asarray(inputs["gamma"], np.float32)
    beta = np.asarray(inputs["beta"], np.float32)
    rmean = np.asarray(inputs["rmean"], np.float32)
    rvar = np.asarray(inputs["rvar"], np.float32)
    Wc, bc = np.asarray(inputs["Wc"], np.float32), np.asarray(inputs["bc"], np.float32)

    src = edge_index[0].astype(np.int64)
    dst = edge_index[1].astype(np.int64)

    # BN folded: A*x + B
    A = gamma / np.sqrt(rvar + EPS)
    B = beta - rmean * A
    bgate = bk + bq + be          # folded into k table
    # rank-blocked row id of the src node in the qv table
    rb_row = (src // NPR) * NPAD + (src % NPR)

    e_rank = dst // NPR
    epoch = (rb_row >= SPLIT).astype(np.int64)
    dst_local = dst - e_rank * NPR
    group = dst_local // 128
    dst_rel = dst_local % 128

    # per (rank, epoch, group) counts -> per-group caps (max over ranks)
    counts = {}
    for ep in (0, 1):
        cnt = np.zeros((R, NGRP), np.int64)
        m = epoch == ep
        np.add.at(cnt, (e_rank[m], group[m]), 1)
        counts[ep] = cnt
    T0g = np.maximum(1, np.ceil(counts[0].max(axis=0) / 128).astype(np.int64))
    T1g = np.maximum(1, np.ceil(counts[1].max(axis=0) / 128).astype(np.int64))
    Tg = T0g + T1g

    # graph counts for mean pooling
    cntg = np.bincount(batch, minlength=G).astype(np.float32)
    inv_cnt = 1.0 / np.maximum(cntg, 1.0)

    # sorted edge arrays: by rank, group, epoch, dst
    order = np.lexsort((dst, epoch, group, e_rank))
    src_s, dst_rel_s, attr_s = rb_row[order], dst_rel[order], edge_attr[order]
    ep_s, rank_s, grp_s = epoch[order], e_rank[order], group[order]
    key = ((rank_s * NGRP + grp_s) * 2 + ep_s)
    starts = np.searchsorted(key, np.arange(R * NGRP * 2 + 1))

    # meta pack layout per group (int16 cols): gidx0 | gidx1 | kidx | dst | attr
    MW = (18 * Tg).astype(np.int64)          # per-group meta width
    moff = np.zeros(NGRP + 1, np.int64)
    np.cumsum(MW, out=moff[1:])
    MTOT = int(moff[-1])

    iota_rep = np.tile(np.arange(128, dtype=np.float32), (128, 1))
    We_rep = np.stack([np.tile(We[l, 0], (128, 1)) for l in range(NLAYER)])
    Wqv = np.concatenate([Wq, Wv], axis=2)   # [L, H, 2H]
    bias_qv = np.zeros((NLAYER, 128, 2 * H), np.float32)
    bias_k = np.zeros((NLAYER, 128, H), np.float32)
    for l in range(NLAYER):
        bias_k[l, :, :] = bgate[l][None, :]
        bias_qv[l, :, H:2 * H] = bv[l][None, :]
    has_bias_qv = bool(np.any(bias_qv != 0))
    has_bias_k = bool(np.any(bias_k != 0))
    has_bs = bool(np.any(bs != 0))
    has_bc = bool(np.any(bc != 0))

    x_rb = np.zeros((R * 128, NPAD), np.float32)
    for r in range(R):
        lo, hi = r * NPR, min((r + 1) * NPR, N)
        x_rb[r * 128:(r + 1) * 128, 0:hi - lo] = x[lo:hi].T

    in_maps = []
    for r in range(R):
        meta_t = np.zeros((128, MTOT), np.int16)
        for g in range(NGRP):
            t0, t1, tg = int(T0g[g]), int(T1g[g]), int(Tg[g])
            o = moff[g]
            dstc = np.full((tg * 128,), -1.0, np.float32)
            attrc = np.zeros((tg * 128,), np.float32)
            kdx = np.zeros((tg * 128,), np.int64)
            for ep, cap, base in ((0, t0, 0), (1, t1, t0)):
                k = (r * NGRP + g) * 2 + ep
                s0 = starts[k]
                n = int(counts[ep][r, g])
                e0 = base * 128
                dstc[e0:e0 + n] = dst_rel_s[s0:s0 + n]
                attrc[e0:e0 + n] = attr_s[s0:s0 + n]
                idx = np.zeros((cap * 128,), np.int64)
                idx[:n] = src_s[s0:s0 + n] - ep * SPLIT
                go = o + (0 if ep == 0 else t0 * 8)
                meta_t[:, go:go + cap * 8] = wrap_idxs_block(idx)
                kdx[e0:e0 + n] = g * 128 + dst_rel_s[s0:s0 + n]
            meta_t[:, o + tg * 8:o + tg * 16] = wrap_idxs_block(kdx)
            meta_t[:, o + tg * 16:o + tg * 17] = (
                colmaj128(dstc).astype(dtab).view(np.int16))
            meta_t[:, o + tg * 17:o + tg * 18] = (
                colmaj128(attrc).astype(dtab).view(np.int16))
        # pooling indicator with 1/cnt folded
        IndT = np.zeros((NPAD, G), np.float32)
        lo, hi = r * NPR, min((r + 1) * NPR, N)
        IndT[np.arange(hi - lo), batch[lo:hi]] = inv_cnt[batch[lo:hi]]
        im = {
            "x_rb": x_rb.astype(dtab),
            "xT_loc": x_rb[r * 128:(r + 1) * 128].astype(dtab),
            "meta": meta_t,
            "Wk": Wk.astype(dtab), "Wqv": Wqv.astype(dtab), "Ws": Ws.astype(dtab),
            "We_rep": We_rep.astype(dtab),
            "bias_qv": bias_qv,
            "bias_k": bias_k,
            "bs_col": bs.reshape(NLAYER, H, 1),
            "A_col": A.reshape(NLAYER, H, 1),
            "B_col": B.reshape(NLAYER, H, 1),
            "iota_rep": iota_rep.astype(dtab),
            "IndT": IndT.astype(dtab),
            "clinT": clinical.T.copy(),
            "Wc_h": Wc[0:H], "Wc_c": Wc[H:H + NCLIN],
            "bc_rep": np.tile(bc, (G, 1)),
        }
        in_maps.append(im)
    meta = dict(T0g=tuple(int(t) for t in T0g), T1g=tuple(int(t) for t in T1g),
                has_bias_qv=has_bias_qv, has_bias_k=has_bias_k,
                has_bs=has_bs, has_bc=has_bc)
    return in_maps, meta


# ---------------------------------------------------------------------------
# device program
# ---------------------------------------------------------------------------

def build(meta):
    T0g, T1g = meta["T0g"], meta["T1g"]
    Tg = [a + b for a, b in zip(T0g, T1g)]
    moff = [0]
    for g in range(NGRP):
        moff.append(moff[-1] + 18 * Tg[g])
    MTOT = moff[-1]
    parts = meta.get("parts", 4)
    DT = BF16

    # chunk column ranges
    chk = []
    g0 = 0
    for ng in CHUNKS:
        chk.append((g0, ng, g0 * 128, ng * 128))
        g0 += ng

    nc = bacc.Bacc("TRN2", target_bir_lowering=False, debug=False, num_devices=R)

    def din(name, shape, dt):
        return nc.dram_tensor(name, shape, dt, kind="ExternalInput").ap()

    t_x_rb = din("x_rb", [R * 128, NPAD], DT)
    t_xT_loc = din("xT_loc", [128, NPAD], DT)
    t_meta = din("meta", [128, MTOT], I16)
    t_Wk = din("Wk", [NLAYER, H, H], DT)
    t_Wqv = din("Wqv", [NLAYER, H, 2 * H], DT)
    t_Ws = din("Ws", [NLAYER, H, H], DT)
    t_We = din("We_rep", [NLAYER, 128, H], DT)
    t_bias_qv = din("bias_qv", [NLAYER, 128, 2 * H], F32)
    t_bias_k = din("bias_k", [NLAYER, 128, H], F32)
    t_bs = din("bs_col", [NLAYER, H, 1], F32)
    t_A = din("A_col", [NLAYER, H, 1], F32)
    t_B = din("B_col", [NLAYER, H, 1], F32)
    t_iota = din("iota_rep", [128, 128], DT)
    t_IndT = din("IndT", [NPAD, G], DT)
    t_clinT = din("clinT", [NCLIN, G], F32)
    t_Wc_h = din("Wc_h", [H, NCLS], F32)
    t_Wc_c = din("Wc_c", [NCLIN, NCLS], F32)
    t_bc = din("bc_rep", [G, NCLS], F32)

    t_out = nc.dram_tensor("out", [G, NCLS], F32, kind="ExternalOutput").ap()

    # double-buffered node tables (parity = layer % 2)
    qv_tab = [nc.dram_tensor(f"qv{p}", [NTOT, 2 * H], DT).ap() for p in range(2)]
    k_tab = [nc.dram_tensor(f"k{p}", [NPAD, H], DT).ap() for p in range(2)]
    # per (layer, chunk) collective buffers
    h_loc = [[nc.dram_tensor(f"hl{l}_{c}", [128, CHUNKS[c] * 128], DT).ap()
              for c in range(len(CHUNKS))] for l in range(2)]
    ag_out = [[nc.dram_tensor(f"ag{l}_{c}", [R * 128, CHUNKS[c] * 128], DT,
                              addr_space="Shared").ap()
               for c in range(len(CHUNKS))] for l in range(2)]
    pool_in = nc.dram_tensor("pool_in", [G, H], F32).ap()
    pool_out = nc.dram_tensor("pool_out", [R * G, H], F32,
                              addr_space="Shared").ap()

    with tile.TileContext(nc) as tc:
        import contextlib
        with contextlib.ExitStack() as ctx:
            consts = ctx.enter_context(tc.tile_pool(name="consts", bufs=1))
            hsb = ctx.enter_context(tc.tile_pool(name="hsb", bufs=1))
            lhp = ctx.enter_context(tc.tile_pool(name="lhp", bufs=3))
            stg = ctx.enter_context(tc.tile_pool(name="stg", bufs=4))
            edg = ctx.enter_context(tc.tile_pool(name="edg", bufs=3))
            edm = ctx.enter_context(tc.tile_pool(name="edm", bufs=3))
            pnode = ctx.enter_context(tc.tile_pool(name="pnode", bufs=3, space="PSUM"))
            pedge = ctx.enter_context(tc.tile_pool(name="pedge", bufs=3, space="PSUM"))
            ppool = ctx.enter_context(tc.tile_pool(name="ppool", bufs=1, space="PSUM"))

            _cid = [0]

            def load_const(src_ap, shape, dt):
                _cid[0] += 1
                t = consts.tile(shape, dt, tag=f"c{_cid[0]}_{src_ap.tensor.name}")
                nc.sync.dma_start(t[:], src_ap)
                return t

            Wk_t = [load_const(t_Wk[l], [H, H], DT) for l in range(NLAYER)]
            Wqv_t = [load_const(t_Wqv[l], [H, 2 * H], DT) for l in range(NLAYER)]
            Ws_t = [load_const(t_Ws[l], [H, H], DT) for l in range(NLAYER)]
            We_t = [load_const(t_We[l], [128, H], DT) for l in range(NLAYER)]
            bias_qv_t = [load_const(t_bias_qv[l], [128, 2 * H], F32)
                         for l in range(NLAYER)] if meta["has_bias_qv"] else None
            bias_k_t = [load_const(t_bias_k[l], [128, H], F32)
                        for l in range(NLAYER)] if meta["has_bias_k"] else None
            bs_t = [load_const(t_bs[l], [H, 1], F32) for l in range(NLAYER)]
            A_t = [load_const(t_A[l], [H, 1], F32) for l in range(NLAYER)]
            B_t = [load_const(t_B[l], [H, 1], F32) for l in range(NLAYER)]
            iota_t = load_const(t_iota, [128, 128], DT)
            ident = consts.tile([128, 128], DT)
            make_identity(nc, ident[:])
            identf = consts.tile([128, 128], F32)
            make_identity(nc, identf[:])
            clin_t = load_const(t_clinT, [NCLIN, G], F32)
            Wch_t = load_const(t_Wc_h, [H, NCLS], F32)
            Wcc_t = load_const(t_Wc_c, [NCLIN, NCLS], F32)
            bc_t = load_const(t_bc, [G, NCLS], F32) if meta["has_bc"] else None

            # double-buffered feature-major accumulators / h tiles
            hs_t = [hsb.tile([128, NPAD], F32, tag=f"hs{p}", name=f"hs{p}")
                    for p in range(2)]
            h3_t = [hsb.tile([128, NPAD], DT, tag=f"h3{p}", name=f"h3{p}")
                    for p in range(2)]

            # ---------------- node-table production helpers ----------------
            def node_sk(l, c0, csz, rhs_src):
                """s-table cols [c0,c0+csz) into hs_t[l%2] + k rows into k_tab.

                rhs_src: feature-major h source; either an SBUF AP (h3 of the
                previous layer) or None (layer 0 -> stream xT_loc).
                """
                hs = hs_t[l % 2]
                for q0 in range(0, csz, 512):
                    qs = min(512, csz - q0)
                    if rhs_src is None:
                        lh = lhp.tile([128, qs], DT, tag="lhx")
                        nc.sync.dma_start(lh[:], t_xT_loc[:, c0 + q0:c0 + q0 + qs])
                        rhs = lh[:]
                    else:
                        rhs = rhs_src[:, c0 + q0:c0 + q0 + qs]
                    ps = pnode.tile([128, 512], F32, tag="pn")
                    nc.tensor.matmul(out=ps[:, 0:qs], lhsT=Ws_t[l][:], rhs=rhs,
                                     start=True, stop=True)
                    if meta["has_bs"]:
                        nc.scalar.activation(hs[:, c0 + q0:c0 + q0 + qs],
                                             ps[:, 0:qs], AF.Identity,
                                             bias=bs_t[l][:], scale=1.0)
                    else:
                        nc.scalar.activation(hs[:, c0 + q0:c0 + q0 + qs],
                                             ps[:, 0:qs], AF.Copy)
                    # k rows for these cols (4 blocks -> one 512-row write)
                    nblk = qs // 128
                    psk = pnode.tile([128, 512], F32, tag="pn")
                    for s in range(nblk):
                        nc.tensor.matmul(
                            out=psk[:, s * 128:(s + 1) * 128],
                            lhsT=rhs[:, s * 128:(s + 1) * 128] if rhs_src is None
                            else rhs_src[:, c0 + q0 + s * 128:c0 + q0 + (s + 1) * 128],
                            rhs=Wk_t[l][:], start=True, stop=True)
                    stk = stg.tile([128, 512], DT, tag="stk")
                    if meta["has_bias_k"]:
                        for s in range(nblk):
                            nc.vector.tensor_tensor(
                                out=stk[:, s * 128:(s + 1) * 128],
                                in0=psk[:, s * 128:(s + 1) * 128],
                                in1=bias_k_t[l][:], op=OP.add)
                    else:
                        nc.scalar.activation(stk[:, 0:nblk * 128],
                                             psk[:, 0:nblk * 128], AF.Copy)
                    dst_ap = k_tab[l % 2][c0 + q0:c0 + q0 + qs, :]
                    nc.sync.dma_start(
                        dst_ap.rearrange("(b p) h -> p b h", p=128),
                        stk[:, 0:nblk * 128])

            def node_qv(l, ci):
                """qv rows for all 8 rank blocks of chunk ci into qv_tab[l%2]."""
                gc0, gng, c0, csz = chk[ci]
                hsrc = t_x_rb if l == 0 else None
                for rb in range(R):
                    if l == 0:
                        src_ap = t_x_rb[rb * 128:(rb + 1) * 128, c0:c0 + csz]
                    else:
                        src_ap = ag_out[(l - 1) % 2][ci][
                            rb * 128:(rb + 1) * 128, :]
                    lh = lhp.tile([128, csz], DT, tag="lh")
                    nc.sync.dma_start(lh[:], src_ap)
                    nblk = csz // 128
                    for s0 in range(0, nblk, 4):
                        nb = min(4, nblk - s0)
                        st = stg.tile([128, 4 * 256], DT, tag="st")
                        for sp in range(0, nb, 2):
                            np2 = min(2, nb - sp)
                            ps = pnode.tile([128, 512], F32, tag="pn")
                            for j in range(np2):
                                s = s0 + sp + j
                                nc.tensor.matmul(
                                    out=ps[:, j * 256:(j + 1) * 256],
                                    lhsT=lh[:, s * 128:(s + 1) * 128],
                                    rhs=Wqv_t[l][:], start=True, stop=True)
                            if meta["has_bias_qv"]:
                                for j in range(np2):
                                    nc.vector.tensor_tensor(
                                        out=st[:, (sp + j) * 256:(sp + j + 1) * 256],
                                        in0=ps[:, j * 256:(j + 1) * 256],
                                        in1=bias_qv_t[l][:], op=OP.add)
                            else:
                                nc.scalar.activation(
                                    st[:, sp * 256:(sp + np2) * 256],
                                    ps[:, 0:np2 * 256], AF.Copy)
                        row = rb * NPAD + c0 + s0 * 128
                        dst_ap = qv_tab[l % 2][row:row + nb * 128, :]
                        nc.sync.dma_start(
                            dst_ap.rearrange("(b p) h -> p b h", p=128),
                            st[:, 0:nb * 256])

            # ---------------- edge phase helper ----------------
            def edge_group(l, g):
                hs = hs_t[l % 2]
                t0, t1, tg = T0g[g], T1g[g], Tg[g]
                o = moff[g]
                mt = edg.tile([128, 18 * tg], I16, tag="meta")
                nc.sync.dma_start(mt[:], t_meta[:, o:o + 18 * tg])
                gt = edg.tile([128, tg, 2 * H], DT, tag="gt")
                nc.gpsimd.dma_gather(
                    gt[:, 0:t0, :], qv_tab[l % 2][0:SPLIT, :],
                    mt[:, 0:t0 * 8], t0 * 128, t0 * 128, 2 * H,
                    single_packet=False)
                nc.gpsimd.dma_gather(
                    gt[:, t0:tg, :], qv_tab[l % 2][SPLIT:NTOT, :],
                    mt[:, t0 * 8:tg * 8], t1 * 128, t1 * 128, 2 * H,
                    single_packet=False)
                kt = edg.tile([128, tg, H], DT, tag="kt")
                nc.gpsimd.dma_gather(
                    kt[:], k_tab[l % 2][:],
                    mt[:, tg * 8:tg * 16], tg * 128, tg * 128, H,
                    single_packet=False)
                dsl = mt[:, tg * 16:tg * 17].bitcast(DT)
                asl = mt[:, tg * 17:tg * 18].bitcast(DT)
                S = edm.tile([128, tg, 128], DT, tag="S")
                nc.vector.tensor_tensor(
                    out=S[:],
                    in0=dsl.unsqueeze(2).to_broadcast([128, tg, 128]),
                    in1=iota_t[:].unsqueeze(1).to_broadcast([128, tg, 128]),
                    op=OP.is_equal)
                nc.vector.tensor_tensor(out=kt[:], in0=kt[:],
                                        in1=gt[:, :, 0:H], op=OP.add)
                et = edm.tile([128, tg, H], DT, tag="et")
                nc.vector.tensor_tensor(
                    out=et[:],
                    in0=asl.unsqueeze(2).to_broadcast([128, tg, H]),
                    in1=We_t[l][:].unsqueeze(1).to_broadcast([128, tg, H]),
                    op=OP.mult)
                nc.vector.tensor_tensor(out=kt[:], in0=kt[:], in1=et[:],
                                        op=OP.add)
                nc.scalar.activation(kt[:], kt[:], AF.Sigmoid)
                nc.vector.tensor_tensor(out=et[:], in0=kt[:],
                                        in1=gt[:, :, H:2 * H], op=OP.mult)
                pa = pedge.tile([128, 128], F32, tag="pa")
                for t in range(tg):
                    nc.tensor.matmul(out=pa[:], lhsT=et[:, t, :],
                                     rhs=S[:, t, :], start=(t == 0),
                                     stop=(t == tg - 1))
                nc.vector.tensor_tensor(
                    out=hs[:, g * 128:(g + 1) * 128],
                    in0=hs[:, g * 128:(g + 1) * 128], in1=pa[:], op=OP.add)

            def bn_chunk(l, ci):
                """leaky+BN for chunk ci cols -> h3_t[l%2]; returns col range."""
                _, _, c0, csz = chk[ci]
                hs, h3 = hs_t[l % 2], h3_t[l % 2]
                nc.scalar.activation(h3[:, c0:c0 + csz], hs[:, c0:c0 + csz],
                                     AF.Lrelu, alpha=SLOPE)
                nc.scalar.activation(h3[:, c0:c0 + csz], h3[:, c0:c0 + csz],
                                     AF.Identity, bias=B_t[l][:],
                                     scale=A_t[l][:])
                return c0, csz

            # ---------------- program ----------------
            # layer 0 node phase (from x)
            for (gc0, gng, c0, csz) in chk:
                node_sk(0, c0, csz, None)
            for ci in range(len(chk)):
                node_qv(0, ci)

            for l in range(NLAYER):
                for ci, (gc0, gng, c0, csz) in enumerate(chk):
                    if parts >= 2:
                        for g in range(gc0, gc0 + gng):
                            edge_group(l, g)
                    if parts < 3:
                        continue
                    c0_, csz_ = bn_chunk(l, ci)
                    if l < NLAYER - 1:
                        h3 = h3_t[l % 2]
                        nc.sync.dma_start(h_loc[l][ci][:], h3[:, c0:c0 + csz])
                        nc.gpsimd.collective_compute(
                            "AllGather", OP.bypass,
                            replica_groups=[list(range(R))],
                            ins=[h_loc[l][ci][:]], outs=[ag_out[l % 2][ci][:]])
                        node_sk(l + 1, c0, csz, h3[:])
                        if ci >= 1:
                            node_qv(l + 1, ci - 1)
                if parts >= 3 and l < NLAYER - 1:
                    node_qv(l + 1, len(chk) - 1)

            if parts < 4:
                z_dbg = stg.tile([G, NCLS], F32, tag="zsb")
                nc.vector.tensor_copy(z_dbg[:], hs_t[0][0:G, 0:NCLS])
                nc.sync.dma_start(t_out[:], z_dbg[:])
            else:
                # ---- pooling over the last layer's h3
                h3f = h3_t[(NLAYER - 1) % 2]
                pp = ppool.tile([G, H], F32)
                for c in range(NGRP):
                    trp = pedge.tile([128, 128], DT, tag="pa")
                    nc.tensor.transpose(out=trp[:], in_=h3f[:, c * 128:(c + 1) * 128],
                                        identity=ident[:])
                    hnode = stg.tile([128, 128], DT, tag="hnode")
                    nc.vector.tensor_copy(hnode[:], trp[:])
                    ind_t = stg.tile([128, G], DT, tag="ind")
                    nc.sync.dma_start(ind_t[:], t_IndT[c * 128:(c + 1) * 128, :])
                    nc.tensor.matmul(out=pp[:], lhsT=ind_t[:], rhs=hnode[:],
                                     start=(c == 0), stop=(c == NGRP - 1))
                pool_sb = stg.tile([G, H], F32, tag="poolsb")
                nc.vector.tensor_copy(pool_sb[:], pp[:])
                nc.sync.dma_start(pool_in[:], pool_sb[:])
                nc.gpsimd.collective_compute(
                    "AllGather", OP.bypass, replica_groups=[list(range(R))],
                    ins=[pool_in[:]], outs=[pool_out[:]])
                # sum the 8 partial pools
                pr = stg.tile([G, R, H], F32, tag="pr")
                nc.sync.dma_start(pr[:], pool_out[:].rearrange("(r g) h -> g r h", r=R))
                pooled = stg.tile([G, H], F32, tag="pooled")
                nc.vector.tensor_tensor(out=pooled[:], in0=pr[:, 0, :], in1=pr[:, 1, :],
                                        op=OP.add)
                for r in range(2, R):
                    nc.vector.tensor_tensor(out=pooled[:], in0=pooled[:],
                                            in1=pr[:, r, :], op=OP.add)
                # transpose pooled [G,H] -> [H,G]
                ptp = pedge.tile([H, G], F32, tag="pa")
                nc.tensor.transpose(out=ptp[:], in_=pooled[:], identity=identf[0:G, 0:G])
                pooledT = stg.tile([H, G], F32, tag="pooledT")
                nc.vector.tensor_copy(pooledT[:], ptp[:])
                zp = pedge.tile([G, NCLS], F32, tag="pa")
                nc.tensor.matmul(out=zp[:], lhsT=pooledT[:], rhs=Wch_t[:],
                                 start=True, stop=False)
                nc.tensor.matmul(out=zp[:], lhsT=clin_t[:], rhs=Wcc_t[:],
                                 start=False, stop=True)
                z_sb = stg.tile([G, NCLS], F32, tag="zsb")
                if meta["has_bc"]:
                    nc.vector.tensor_tensor(out=z_sb[:], in0=zp[:], in1=bc_t[:],
                                            op=OP.add)
                else:
                    nc.vector.tensor_copy(z_sb[:], zp[:])
                nc.sync.dma_start(t_out[:], z_sb[:])

    nc.compile()
    return nc


# ---------------------------------------------------------------------------

_CACHE = {}


def kernel(**inputs):
    in_maps, meta = prep(inputs)
    key = tuple(sorted((k, v) for k, v in meta.items()))
    if key not in _CACHE:
        _CACHE[key] = build(meta)
    nc = _CACHE[key]
    res = run_bass_kernel_spmd(nc, in_maps, list(range(R)))
    return np.asarray(res.results[0]["out"], np.float32)


def kernel_profiled(**inputs):
    """Like kernel() but also returns (exec_time_ns, trace_path)."""
    in_maps, meta = prep(inputs)
    key = tuple(sorted((k, v) for k, v in meta.items()))
    if key not in _CACHE:
        _CACHE[key] = build(meta)
    nc = _CACHE[key]
    res = run_bass_kernel_spmd(nc, in_maps, list(range(R)), trace=True)
    out = np.asarray(res.results[0]["out"], np.float32)
    trace_path = None
    if res.instructions_and_trace is not None:
        trace_path = res.instructions_and_trace[1]
    return out, res.exec_time_ns, trace_path


if __name__ == "__main__":
    pass


# revision 13
# speedup vs baseline: 2.2210x; 2.2210x over previous
"""Trainium2 Bass kernel for a 3-layer ResGatedGraphConv GNN (ClinicalGatedGCN).

Strategy (8 NeuronCores, SPMD):
  - Nodes are partitioned into 8 contiguous ranges (rank-blocked ids, padded to
    a multiple of 128 per rank). Edges are assigned to the rank that owns their
    dst node, grouped by 128-wide dst blocks, and sorted by (epoch, dst) on the
    host (epoch = whether the src row id fits the int16 gather index range).
  - Each rank computes the full [q|v] node table (node-major, one matmul per
    128-node block with the h chunk as the stationary operand, N=256) into a
    single local HBM table, plus a rank-local k table indexed by dst. Per edge
    group one dma_gather per epoch fetches [q|v] src rows into one tile, and a
    single merged gather fetches k[dst] for both epochs.
  - Gate math runs once per group over both epochs' slots in packed layouts
    (DVE 2x mode); segment-sum over dst is a PE matmul against a 0/1 selector
    built on-device with is_equal; both epochs accumulate in one PSUM group.
  - h updates (leaky+BN) run on the scalar engine per AllGather chunk; the
    per-layer h AllGather is split into 4 chunks issued inside the edge loop so
    the collective overlaps edge processing, and the next layer's s/k/qv table
    production is interleaved per chunk to hide the node phase.
  - Mean-pool per graph is a matmul against a host-built indicator with 1/cnt
    folded in; partial pools are AllGather'd and summed; the tiny classifier
    runs on every core.
"""

import numpy as np
import ml_dtypes

import concourse.bacc as bacc
import concourse.bass as bass
import concourse.mybir as mybir
import concourse.tile as tile
from concourse.bass_utils import run_bass_kernel_spmd
from concourse.masks import make_identity

F32 = mybir.dt.float32
BF16 = mybir.dt.bfloat16
F8 = mybir.dt.float8e4
I16 = mybir.dt.int16
AF = mybir.ActivationFunctionType
OP = mybir.AluOpType

# ---------------- problem constants (hardcoded per spec) ----------------
N, E, H, G, NCLIN, NCLS = 50000, 800000, 128, 64, 16, 2
NLAYER = 3
EPS = 1e-5
SLOPE = 0.01
R = 8                      # ranks / NeuronCores
SPLIT = 32768              # int16 gather index limit -> 2 epochs

NPR = (N + R - 1) // R     # real nodes per rank
NGRP = (NPR + 127) // 128  # 128-node groups per rank
NPAD = NGRP * 128          # padded nodes per rank
NTOT = R * NPAD            # rank-blocked total rows

CHUNKS = [16, 16, 12, 5]   # edge groups per AllGather chunk
assert sum(CHUNKS) == NGRP


def wrap_idxs_block(idx):
    """Wrap one gather call's indices: idx j -> [j%16, j//16], tiled to 128 parts."""
    n = len(idx)
    assert n % 16 == 0
    w = np.asarray(idx, np.int16).reshape(n // 16, 16).T
    return np.tile(w, (8, 1))


def colmaj128(v):
    """Edge-scalar array -> [128, n/128] layout (edge j at [j%128, j//128])."""
    v = np.asarray(v)
    n = v.shape[0]
    assert n % 128 == 0
    return v.reshape(n // 128, 128).T.copy()


# ---------------------------------------------------------------------------
# host-side preprocessing
# ---------------------------------------------------------------------------

def prep(inputs):
    dtab = ml_dtypes.bfloat16
    x = np.asarray(inputs["x"], np.float32)
    edge_index = np.asarray(inputs["edge_index"])
    edge_attr = np.asarray(inputs["edge_attr"], np.float32)[:, 0]
    batch = np.asarray(inputs["batch"]).astype(np.int64)
    clinical = np.asarray(inputs["clinical"], np.float32)
    Wk, bk = np.asarray(inputs["Wk"], np.float32), np.asarray(inputs["bk"], np.float32)
    Wq, bq = np.asarray(inputs["Wq"], np.float32), np.asarray(inputs["bq"], np.float32)
    Wv, bv = np.asarray(inputs["Wv"], np.float32), np.asarray(inputs["bv"], np.float32)
    Ws, bs = np.asarray(inputs["Ws"], np.float32), np.asarray(inputs["bs"], np.float32)
    We, be = np.asarray(inputs["We"], np.float32), np.asarray(inputs["be"], np.float32)
    gamma = np.asarray(inputs["gamma"], np.float32)
    beta = np.asarray(inputs["beta"], np.float32)
    rmean = np.asarray(inputs["rmean"], np.float32)
    rvar = np.asarray(inputs["rvar"], np.float32)
    Wc, bc = np.asarray(inputs["Wc"], np.float32), np.asarray(inputs["bc"], np.float32)

    src = edge_index[0].astype(np.int64)
    dst = edge_index[1].astype(np.int64)

    # BN folded: A*x + B
    A = gamma / np.sqrt(rvar + EPS)
    B = beta - rmean * A
    bgate = bk + bq + be          # folded into k table
    # rank-blocked row id of the src node in the qv table
    rb_row = (src // NPR) * NPAD + (src % NPR)

    e_rank = dst // NPR
    epoch = (rb_row >= SPLIT).astype(np.int64)
    dst_local = dst - e_rank * NPR
    group = dst_local // 128
    dst_rel = dst_local % 128

    # per (rank, epoch, group) counts -> per-group caps (max over ranks)
    counts = {}
    for ep in (0, 1):
        cnt = np.zeros((R, NGRP), np.int64)
        m = epoch == ep
        np.add.at(cnt, (e_rank[m], group[m]), 1)
        counts[ep] = cnt
    T0g = np.maximum(1, np.ceil(counts[0].max(axis=0) / 128).astype(np.int64))
    T1g = np.maximum(1, np.ceil(counts[1].max(axis=0) / 128).astype(np.int64))
    Tg = T0g + T1g

    # graph counts for mean pooling
    cntg = np.bincount(batch, minlength=G).astype(np.float32)
    inv_cnt = 1.0 / np.maximum(cntg, 1.0)

    # sorted edge arrays: by rank, group, epoch, dst
    order = np.lexsort((dst, epoch, group, e_rank))
    src_s, dst_rel_s, attr_s = rb_row[order], dst_rel[order], edge_attr[order]
    ep_s, rank_s, grp_s = epoch[order], e_rank[order], group[order]
    key = ((rank_s * NGRP + grp_s) * 2 + ep_s)
    starts = np.searchsorted(key, np.arange(R * NGRP * 2 + 1))

    # meta pack layout per group (int16 cols): gidx0 | gidx1 | kidx | dst | attr
    MW = (18 * Tg).astype(np.int64)          # per-group meta width
    moff = np.zeros(NGRP + 1, np.int64)
    np.cumsum(MW, out=moff[1:])
    MTOT = int(moff[-1])

    iota_rep = np.tile(np.arange(128, dtype=np.float32), (128, 1))
    We_rep = np.stack([np.tile(We[l, 0], (128, 1)) for l in range(NLAYER)])
    Wqv = np.concatenate([Wq, Wv], axis=2)   # [L, H, 2H]
    bias_qv = np.zeros((NLAYER, 128, 2 * H), np.float32)
    bias_k = np.zeros((NLAYER, 128, H), np.float32)
    for l in range(NLAYER):
        bias_k[l, :, :] = bgate[l][None, :]
        bias_qv[l, :, H:2 * H] = bv[l][None, :]
    has_bias_qv = bool(np.any(bias_qv != 0))
    has_bias_k = bool(np.any(bias_k != 0))
    has_bs = bool(np.any(bs != 0))
    has_bc = bool(np.any(bc != 0))

    x_rb = np.zeros((R * 128, NPAD), np.float32)
    for r in range(R):
        lo, hi = r * NPR, min((r + 1) * NPR, N)
        x_rb[r * 128:(r + 1) * 128, 0:hi - lo] = x[lo:hi].T

    in_maps = []
    for r in range(R):
        meta_t = np.zeros((128, MTOT), np.int16)
        for g in range(NGRP):
            t0, t1, tg = int(T0g[g]), int(T1g[g]), int(Tg[g])
            o = moff[g]
            dstc = np.full((tg * 128,), -1.0, np.float32)
            attrc = np.zeros((tg * 128,), np.float32)
            kdx = np.zeros((tg * 128,), np.int64)
            for ep, cap, base in ((0, t0, 0), (1, t1, t0)):
                k = (r * NGRP + g) * 2 + ep
                s0 = starts[k]
                n = int(counts[ep][r, g])
                e0 = base * 128
                dstc[e0:e0 + n] = dst_rel_s[s0:s0 + n]
                attrc[e0:e0 + n] = attr_s[s0:s0 + n]
                idx = np.zeros((cap * 128,), np.int64)
                idx[:n] = src_s[s0:s0 + n] - ep * SPLIT
                go = o + (0 if ep == 0 else t0 * 8)
                meta_t[:, go:go + cap * 8] = wrap_idxs_block(idx)
                kdx[e0:e0 + n] = g * 128 + dst_rel_s[s0:s0 + n]
            meta_t[:, o + tg * 8:o + tg * 16] = wrap_idxs_block(kdx)
            meta_t[:, o + tg * 16:o + tg * 17] = (
                colmaj128(dstc).astype(dtab).view(np.int16))
            meta_t[:, o + tg * 17:o + tg * 18] = (
                colmaj128(attrc).astype(dtab).view(np.int16))
        # pooling indicator with 1/cnt folded
        IndT = np.zeros((NPAD, G), np.float32)
        lo, hi = r * NPR, min((r + 1) * NPR, N)
        IndT[np.arange(hi - lo), batch[lo:hi]] = inv_cnt[batch[lo:hi]]
        im = {
            "x_rb": x_rb.astype(ml_dtypes.float8_e4m3),
            "xT_loc": x_rb[r * 128:(r + 1) * 128].astype(dtab),
            "meta": meta_t,
            "Wk": Wk.astype(dtab), "Wqv": Wqv.astype(dtab), "Ws": Ws.astype(dtab),
            "We_rep": We_rep.astype(dtab),
            "bias_qv": bias_qv,
            "bias_k": bias_k,
            "bs_col": bs.reshape(NLAYER, H, 1),
            "A_col": A.reshape(NLAYER, H, 1),
            "B_col": B.reshape(NLAYER, H, 1),
            "iota_rep": iota_rep.astype(dtab),
            "IndT": IndT.astype(dtab),
            "clinT": clinical.T.copy(),
            "Wc_h": Wc[0:H], "Wc_c": Wc[H:H + NCLIN],
            "bc_rep": np.tile(bc, (G, 1)),
        }
        in_maps.append(im)
    meta = dict(T0g=tuple(int(t) for t in T0g), T1g=tuple(int(t) for t in T1g),
                has_bias_qv=has_bias_qv, has_bias_k=has_bias_k,
                has_bs=has_bs, has_bc=has_bc)
    return in_maps, meta


# ---------------------------------------------------------------------------
# device program
# ---------------------------------------------------------------------------

def build(meta):
    T0g, T1g = meta["T0g"], meta["T1g"]
    Tg = [a + b for a, b in zip(T0g, T1g)]
    moff = [0]
    for g in range(NGRP):
        moff.append(moff[-1] + 18 * Tg[g])
    MTOT = moff[-1]
    parts = meta.get("parts", 4)
    DT = BF16

    # chunk column ranges
    chk = []
    g0 = 0
    for ng in CHUNKS:
        chk.append((g0, ng, g0 * 128, ng * 128))
        g0 += ng

    nc = bacc.Bacc("TRN2", target_bir_lowering=False, debug=False, num_devices=R)

    def din(name, shape, dt):
        return nc.dram_tensor(name, shape, dt, kind="ExternalInput").ap()

    t_x_rb = din("x_rb", [R * 128, NPAD], F8)
    t_xT_loc = din("xT_loc", [128, NPAD], DT)
    t_meta = din("meta", [128, MTOT], I16)
    t_Wk = din("Wk", [NLAYER, H, H], DT)
    t_Wqv = din("Wqv", [NLAYER, H, 2 * H], DT)
    t_Ws = din("Ws", [NLAYER, H, H], DT)
    t_We = din("We_rep", [NLAYER, 128, H], DT)
    t_bias_qv = din("bias_qv", [NLAYER, 128, 2 * H], F32)
    t_bias_k = din("bias_k", [NLAYER, 128, H], F32)
    t_bs = din("bs_col", [NLAYER, H, 1], F32)
    t_A = din("A_col", [NLAYER, H, 1], F32)
    t_B = din("B_col", [NLAYER, H, 1], F32)
    t_iota = din("iota_rep", [128, 128], DT)
    t_IndT = din("IndT", [NPAD, G], DT)
    t_clinT = din("clinT", [NCLIN, G], F32)
    t_Wc_h = din("Wc_h", [H, NCLS], F32)
    t_Wc_c = din("Wc_c", [NCLIN, NCLS], F32)
    t_bc = din("bc_rep", [G, NCLS], F32)

    t_out = nc.dram_tensor("out", [G, NCLS], F32, kind="ExternalOutput").ap()

    # double-buffered node tables (parity = layer % 2)
    qv_tab = [nc.dram_tensor(f"qv{p}", [NTOT, 2 * H], DT).ap() for p in range(2)]
    k_tab = [nc.dram_tensor(f"k{p}", [NPAD, H], DT).ap() for p in range(2)]
    # per (layer, chunk) collective buffers
    h_loc = [[nc.dram_tensor(f"hl{l}_{c}", [128, CHUNKS[c] * 128], F8).ap()
              for c in range(len(CHUNKS))] for l in range(2)]
    ag_out = [[nc.dram_tensor(f"ag{l}_{c}", [R * 128, CHUNKS[c] * 128], F8,
                              addr_space="Shared").ap()
               for c in range(len(CHUNKS))] for l in range(2)]
    pool_in = nc.dram_tensor("pool_in", [G, H], F32).ap()
    pool_out = nc.dram_tensor("pool_out", [R * G, H], F32,
                              addr_space="Shared").ap()

    with tile.TileContext(nc) as tc:
        import contextlib
        with contextlib.ExitStack() as ctx:
            consts = ctx.enter_context(tc.tile_pool(name="consts", bufs=1))
            hsb = ctx.enter_context(tc.tile_pool(name="hsb", bufs=1))
            lhp = ctx.enter_context(tc.tile_pool(name="lhp", bufs=2))
            stg = ctx.enter_context(tc.tile_pool(name="stg", bufs=3))
            edg = ctx.enter_context(tc.tile_pool(name="edg", bufs=3))
            edm = ctx.enter_context(tc.tile_pool(name="edm", bufs=2))
            pnode = ctx.enter_context(tc.tile_pool(name="pnode", bufs=3, space="PSUM"))
            pedge = ctx.enter_context(tc.tile_pool(name="pedge", bufs=3, space="PSUM"))
            ppool = ctx.enter_context(tc.tile_pool(name="ppool", bufs=1, space="PSUM"))

            _cid = [0]

            def load_const(src_ap, shape, dt):
                _cid[0] += 1
                t = consts.tile(shape, dt, tag=f"c{_cid[0]}_{src_ap.tensor.name}")
                nc.sync.dma_start(t[:], src_ap)
                return t

            Wk_t = [load_const(t_Wk[l], [H, H], DT) for l in range(NLAYER)]
            Wqv_t = [load_const(t_Wqv[l], [H, 2 * H], DT) for l in range(NLAYER)]
            Ws_t = [load_const(t_Ws[l], [H, H], DT) for l in range(NLAYER)]
            We_t = [load_const(t_We[l], [128, H], DT) for l in range(NLAYER)]
            bias_qv_t = [load_const(t_bias_qv[l], [128, 2 * H], F32)
                         for l in range(NLAYER)] if meta["has_bias_qv"] else None
            bias_k_t = [load_const(t_bias_k[l], [128, H], F32)
                        for l in range(NLAYER)] if meta["has_bias_k"] else None
            bs_t = [load_const(t_bs[l], [H, 1], F32) for l in range(NLAYER)]
            A_t = [load_const(t_A[l], [H, 1], F32) for l in range(NLAYER)]
            B_t = [load_const(t_B[l], [H, 1], F32) for l in range(NLAYER)]
            iota_t = load_const(t_iota, [128, 128], DT)
            ident = consts.tile([128, 128], DT)
            make_identity(nc, ident[:])
            identf = consts.tile([128, 128], F32)
            make_identity(nc, identf[:])
            clin_t = load_const(t_clinT, [NCLIN, G], F32)
            Wch_t = load_const(t_Wc_h, [H, NCLS], F32)
            Wcc_t = load_const(t_Wc_c, [NCLIN, NCLS], F32)
            bc_t = load_const(t_bc, [G, NCLS], F32) if meta["has_bc"] else None

            # double-buffered feature-major accumulators / h tiles
            hs_t = [hsb.tile([128, NPAD], F32, tag=f"hs{p}", name=f"hs{p}")
                    for p in range(2)]
            h3_t = [hsb.tile([128, NPAD], DT, tag=f"h3{p}", name=f"h3{p}")
                    for p in range(2)]

            # ---------------- node-table production helpers ----------------
            def node_sk(l, c0, csz, rhs_src):
                """s-table cols [c0,c0+csz) into hs_t[l%2] + k rows into k_tab.

                rhs_src: feature-major h source; either an SBUF AP (h3 of the
                previous layer) or None (layer 0 -> stream xT_loc).
                """
                hs = hs_t[l % 2]
                for q0 in range(0, csz, 512):
                    qs = min(512, csz - q0)
                    if rhs_src is None:
                        lh = lhp.tile([128, qs], DT, tag="lhx")
                        nc.sync.dma_start(lh[:], t_xT_loc[:, c0 + q0:c0 + q0 + qs])
                        rhs = lh[:]
                    else:
                        rhs = rhs_src[:, c0 + q0:c0 + q0 + qs]
                    ps = pnode.tile([128, 512], F32, tag="pn")
                    nc.tensor.matmul(out=ps[:, 0:qs], lhsT=Ws_t[l][:], rhs=rhs,
                                     start=True, stop=True)
                    if meta["has_bs"]:
                        nc.scalar.activation(hs[:, c0 + q0:c0 + q0 + qs],
                                             ps[:, 0:qs], AF.Identity,
                                             bias=bs_t[l][:], scale=1.0)
                    else:
                        nc.scalar.activation(hs[:, c0 + q0:c0 + q0 + qs],
                                             ps[:, 0:qs], AF.Copy)
                    # k rows for these cols (4 blocks -> one 512-row write)
                    nblk = qs // 128
                    psk = pnode.tile([128, 512], F32, tag="pn")
                    for s in range(nblk):
                        nc.tensor.matmul(
                            out=psk[:, s * 128:(s + 1) * 128],
                            lhsT=rhs[:, s * 128:(s + 1) * 128] if rhs_src is None
                            else rhs_src[:, c0 + q0 + s * 128:c0 + q0 + (s + 1) * 128],
                            rhs=Wk_t[l][:], start=True, stop=True)
                    stk = stg.tile([128, 512], DT, tag="stk")
                    if meta["has_bias_k"]:
                        for s in range(nblk):
                            nc.vector.tensor_tensor(
                                out=stk[:, s * 128:(s + 1) * 128],
                                in0=psk[:, s * 128:(s + 1) * 128],
                                in1=bias_k_t[l][:], op=OP.add)
                    else:
                        nc.scalar.activation(stk[:, 0:nblk * 128],
                                             psk[:, 0:nblk * 128], AF.Copy)
                    dst_ap = k_tab[l % 2][c0 + q0:c0 + q0 + qs, :]
                    nc.sync.dma_start(
                        dst_ap.rearrange("(b p) h -> p b h", p=128),
                        stk[:, 0:nblk * 128])

            def node_qv(l, ci):
                """qv rows for all 8 rank blocks of chunk ci into qv_tab[l%2]."""
                gc0, gng, c0, csz = chk[ci]
                hsrc = t_x_rb if l == 0 else None
                for rb in range(R):
                    if l == 0:
                        src_ap = t_x_rb[rb * 128:(rb + 1) * 128, c0:c0 + csz]
                    else:
                        src_ap = ag_out[(l - 1) % 2][ci][
                            rb * 128:(rb + 1) * 128, :]
                    lh8 = lhp.tile([128, csz], F8, tag="lh8")
                    nc.sync.dma_start(lh8[:], src_ap)
                    lh = lhp.tile([128, csz], DT, tag="lh")
                    nc.scalar.activation(lh[:], lh8[:], AF.Copy)
                    nblk = csz // 128
                    for s0 in range(0, nblk, 4):
                        nb = min(4, nblk - s0)
                        st = stg.tile([128, 4 * 256], DT, tag="st")
                        for sp in range(0, nb, 2):
                            np2 = min(2, nb - sp)
                            ps = pnode.tile([128, 512], F32, tag="pn")
                            for j in range(np2):
                                s = s0 + sp + j
                                nc.tensor.matmul(
                                    out=ps[:, j * 256:(j + 1) * 256],
                                    lhsT=lh[:, s * 128:(s + 1) * 128],
                                    rhs=Wqv_t[l][:], start=True, stop=True)
                            if meta["has_bias_qv"]:
                                for j in range(np2):
                                    nc.vector.tensor_tensor(
                                        out=st[:, (sp + j) * 256:(sp + j + 1) * 256],
                                        in0=ps[:, j * 256:(j + 1) * 256],
                                        in1=bias_qv_t[l][:], op=OP.add)
                            else:
                                nc.scalar.activation(
                                    st[:, sp * 256:(sp + np2) * 256],
                                    ps[:, 0:np2 * 256], AF.Copy)
                        row = rb * NPAD + c0 + s0 * 128
                        dst_ap = qv_tab[l % 2][row:row + nb * 128, :]
                        nc.sync.dma_start(
                            dst_ap.rearrange("(b p) h -> p b h", p=128),
                            st[:, 0:nb * 256])

            # ---------------- edge phase helper ----------------
            def edge_group(l, g):
                hs = hs_t[l % 2]
                t0, t1, tg = T0g[g], T1g[g], Tg[g]
                o = moff[g]
                mt = edg.tile([128, 18 * tg], I16, tag="meta")
                nc.sync.dma_start(mt[:], t_meta[:, o:o + 18 * tg])
                gt = edg.tile([128, tg, 2 * H], DT, tag="gt")
                nc.gpsimd.dma_gather(
                    gt[:, 0:t0, :], qv_tab[l % 2][0:SPLIT, :],
                    mt[:, 0:t0 * 8], t0 * 128, t0 * 128, 2 * H,
                    single_packet=False)
                nc.gpsimd.dma_gather(
                    gt[:, t0:tg, :], qv_tab[l % 2][SPLIT:NTOT, :],
                    mt[:, t0 * 8:tg * 8], t1 * 128, t1 * 128, 2 * H,
                    single_packet=False)
                kt = edg.tile([128, tg, H], DT, tag="kt")
                nc.gpsimd.dma_gather(
                    kt[:], k_tab[l % 2][:],
                    mt[:, tg * 8:tg * 16], tg * 128, tg * 128, H,
                    single_packet=False)
                dsl = mt[:, tg * 16:tg * 17].bitcast(DT)
                asl = mt[:, tg * 17:tg * 18].bitcast(DT)
                S = edm.tile([128, tg, 128], DT, tag="S")
                nc.vector.tensor_tensor(
                    out=S[:],
                    in0=dsl.unsqueeze(2).to_broadcast([128, tg, 128]),
                    in1=iota_t[:].unsqueeze(1).to_broadcast([128, tg, 128]),
                    op=OP.is_equal)
                nc.vector.tensor_tensor(out=kt[:], in0=kt[:],
                                        in1=gt[:, :, 0:H], op=OP.add)
                et = edm.tile([128, tg, H], DT, tag="et")
                nc.vector.tensor_tensor(
                    out=et[:],
                    in0=asl.unsqueeze(2).to_broadcast([128, tg, H]),
                    in1=We_t[l][:].unsqueeze(1).to_broadcast([128, tg, H]),
                    op=OP.mult)
                nc.vector.tensor_tensor(out=kt[:], in0=kt[:], in1=et[:],
                                        op=OP.add)
                nc.scalar.activation(kt[:], kt[:], AF.Sigmoid)
                nc.vector.tensor_tensor(out=et[:], in0=kt[:],
                                        in1=gt[:, :, H:2 * H], op=OP.mult)
                pa = pedge.tile([128, 128], F32, tag="pa")
                for t in range(tg):
                    nc.tensor.matmul(out=pa[:], lhsT=et[:, t, :],
                                     rhs=S[:, t, :], start=(t == 0),
                                     stop=(t == tg - 1))
                nc.vector.tensor_tensor(
                    out=hs[:, g * 128:(g + 1) * 128],
                    in0=hs[:, g * 128:(g + 1) * 128], in1=pa[:], op=OP.add)

            def bn_chunk(l, ci):
                """leaky+BN for chunk ci cols -> h3_t[l%2]; returns col range."""
                _, _, c0, csz = chk[ci]
                hs, h3 = hs_t[l % 2], h3_t[l % 2]
                nc.scalar.activation(h3[:, c0:c0 + csz], hs[:, c0:c0 + csz],
                                     AF.Lrelu, alpha=SLOPE)
                nc.scalar.activation(h3[:, c0:c0 + csz], h3[:, c0:c0 + csz],
                                     AF.Identity, bias=B_t[l][:],
                                     scale=A_t[l][:])
                return c0, csz

            # ---------------- program ----------------
            # layer 0 node phase (from x)
            for (gc0, gng, c0, csz) in chk:
                node_sk(0, c0, csz, None)
            for ci in range(len(chk)):
                node_qv(0, ci)

            for l in range(NLAYER):
                for ci, (gc0, gng, c0, csz) in enumerate(chk):
                    if parts >= 2:
                        for g in range(gc0, gc0 + gng):
                            edge_group(l, g)
                    if parts < 3:
                        continue
                    c0_, csz_ = bn_chunk(l, ci)
                    if l < NLAYER - 1:
                        h3 = h3_t[l % 2]
                        h8 = stg.tile([128, csz], F8, tag="h8")
                        nc.scalar.activation(h8[:], h3[:, c0:c0 + csz], AF.Copy)
                        nc.sync.dma_start(h_loc[l][ci][:], h8[:])
                        nc.gpsimd.collective_compute(
                            "AllGather", OP.bypass,
                            replica_groups=[list(range(R))],
                            ins=[h_loc[l][ci][:]], outs=[ag_out[l % 2][ci][:]])
                        node_sk(l + 1, c0, csz, h3[:])
                        if ci >= 1:
                            node_qv(l + 1, ci - 1)
                if parts >= 3 and l < NLAYER - 1:
                    node_qv(l + 1, len(chk) - 1)

            if parts < 4:
                z_dbg = stg.tile([G, NCLS], F32, tag="zsb")
                nc.vector.tensor_copy(z_dbg[:], hs_t[0][0:G, 0:NCLS])
                nc.sync.dma_start(t_out[:], z_dbg[:])
            else:
                # ---- pooling over the last layer's h3
                h3f = h3_t[(NLAYER - 1) % 2]
                pp = ppool.tile([G, H], F32)
                for c in range(NGRP):
                    trp = pedge.tile([128, 128], DT, tag="pa")
                    nc.tensor.transpose(out=trp[:], in_=h3f[:, c * 128:(c + 1) * 128],
                                        identity=ident[:])
                    hnode = stg.tile([128, 128], DT, tag="hnode")
                    nc.vector.tensor_copy(hnode[:], trp[:])
                    ind_t = stg.tile([128, G], DT, tag="ind")
                    nc.sync.dma_start(ind_t[:], t_IndT[c * 128:(c + 1) * 128, :])
                    nc.tensor.matmul(out=pp[:], lhsT=ind_t[:], rhs=hnode[:],
                                     start=(c == 0), stop=(c == NGRP - 1))
                pool_sb = stg.tile([G, H], F32, tag="poolsb")
                nc.vector.tensor_copy(pool_sb[:], pp[:])
                nc.sync.dma_start(pool_in[:], pool_sb[:])
                nc.gpsimd.collective_compute(
                    "AllGather", OP.bypass, replica_groups=[list(range(R))],
                    ins=[pool_in[:]], outs=[pool_out[:]])
                # sum the 8 partial pools
                pr = stg.tile([G, R, H], F32, tag="pr")
                nc.sync.dma_start(pr[:], pool_out[:].rearrange("(r g) h -> g r h", r=R))
                pooled = stg.tile([G, H], F32, tag="pooled")
                nc.vector.tensor_tensor(out=pooled[:], in0=pr[:, 0, :], in1=pr[:, 1, :],
                                        op=OP.add)
                for r in range(2, R):
                    nc.vector.tensor_tensor(out=pooled[:], in0=pooled[:],
                                            in1=pr[:, r, :], op=OP.add)
                # transpose pooled [G,H] -> [H,G]
                ptp = pedge.tile([H, G], F32, tag="pa")
                nc.tensor.transpose(out=ptp[:], in_=pooled[:], identity=identf[0:G, 0:G])
                pooledT = stg.tile([H, G], F32, tag="pooledT")
                nc.vector.tensor_copy(pooledT[:], ptp[:])
                zp = pedge.tile([G, NCLS], F32, tag="pa")
                nc.tensor.matmul(out=zp[:], lhsT=pooledT[:], rhs=Wch_t[:],
                                 start=True, stop=False)
                nc.tensor.matmul(out=zp[:], lhsT=clin_t[:], rhs=Wcc_t[:],
                                 start=False, stop=True)
                z_sb = stg.tile([G, NCLS], F32, tag="zsb")
                if meta["has_bc"]:
                    nc.vector.tensor_tensor(out=z_sb[:], in0=zp[:], in1=bc_t[:],
                                            op=OP.add)
                else:
                    nc.vector.tensor_copy(z_sb[:], zp[:])
                nc.sync.dma_start(t_out[:], z_sb[:])

    nc.compile()
    return nc


# ---------------------------------------------------------------------------

_CACHE = {}


def kernel(**inputs):
    in_maps, meta = prep(inputs)
    key = tuple(sorted((k, v) for k, v in meta.items()))
    if key not in _CACHE:
        _CACHE[key] = build(meta)
    nc = _CACHE[key]
    res = run_bass_kernel_spmd(nc, in_maps, list(range(R)))
    return np.asarray(res.results[0]["out"], np.float32)


def kernel_profiled(**inputs):
    """Like kernel() but also returns (exec_time_ns, trace_path)."""
    in_maps, meta = prep(inputs)
    key = tuple(sorted((k, v) for k, v in meta.items()))
    if key not in _CACHE:
        _CACHE[key] = build(meta)
    nc = _CACHE[key]
    res = run_bass_kernel_spmd(nc, in_maps, list(range(R)), trace=True)
    out = np.asarray(res.results[0]["out"], np.float32)
    trace_path = None
    if res.instructions_and_trace is not None:
        trace_path = res.instructions_and_trace[1]
    return out, res.exec_time_ns, trace_path


if __name__ == "__main__":
    pass


# revision 30
# speedup vs baseline: 2.2375x; 1.0074x over previous
"""Trainium2 Bass kernel for a 3-layer ResGatedGraphConv GNN (ClinicalGatedGCN).

Strategy (8 NeuronCores, SPMD):
  - Nodes are partitioned into 8 contiguous ranges (rank-blocked ids, padded to
    a multiple of 128 per rank). Edges are assigned to the rank that owns their
    dst node, grouped by 128-wide dst blocks, and sorted by (epoch, dst) on the
    host (epoch = whether the src row id fits the int16 gather index range).
  - Each rank computes the full [q|v] node table (node-major, one matmul per
    128-node block with the h chunk as the stationary operand, N=256) into a
    single local HBM table, plus a rank-local k table indexed by dst. Per edge
    group one dma_gather per epoch fetches [q|v] src rows into one tile, and a
    single merged gather fetches k[dst] for both epochs.
  - Gate math runs once per group over both epochs' slots in packed layouts
    (DVE 2x mode); segment-sum over dst is a PE matmul against a 0/1 selector
    built on-device with is_equal; both epochs accumulate in one PSUM group.
  - h updates (leaky+BN) run on the scalar engine per AllGather chunk; the
    per-layer h AllGather is split into 4 chunks issued inside the edge loop so
    the collective overlaps edge processing, and the next layer's s/k/qv table
    production is interleaved per chunk to hide the node phase.
  - Mean-pool per graph is a matmul against a host-built indicator with 1/cnt
    folded in; partial pools are AllGather'd and summed; the tiny classifier
    runs on every core.
"""

import numpy as np
import ml_dtypes

import concourse.bacc as bacc
import concourse.bass as bass
import concourse.mybir as mybir
import concourse.tile as tile
from concourse.bass_utils import run_bass_kernel_spmd
from concourse.masks import make_identity

F32 = mybir.dt.float32
BF16 = mybir.dt.bfloat16
F8 = mybir.dt.float8e4
I16 = mybir.dt.int16
AF = mybir.ActivationFunctionType
OP = mybir.AluOpType

# ---------------- problem constants (hardcoded per spec) ----------------
N, E, H, G, NCLIN, NCLS = 50000, 800000, 128, 64, 16, 2
NLAYER = 3
EPS = 1e-5
SLOPE = 0.01
R = 8                      # ranks / NeuronCores
SPLIT = 32768              # int16 gather index limit -> 2 epochs

NPR = (N + R - 1) // R     # real nodes per rank
NGRP = (NPR + 127) // 128  # 128-node groups per rank
NPAD = NGRP * 128          # padded nodes per rank
NTOT = R * NPAD            # rank-blocked total rows

CHUNKS = [16, 16, 12, 5]   # edge groups per AllGather chunk
assert sum(CHUNKS) == NGRP

# scheduler pins (ms of sim time) for the AG-dependent qv-table production:
# without these the list scheduler hoists the ag_out loads right behind the
# collective's issue point, and their semaphore wait head-of-line blocks the
# sync-DMA queue (meta loads) for the whole collective duration. The defaults
# are refined by a pilot build + timeline sim in build_tuned().
QV_WAITS = {(1, 0): 0.46, (1, 1): 0.61, (1, 2): 0.70, (1, 3): 0.76,
            (2, 0): 1.10, (2, 1): 1.25, (2, 2): 1.34, (2, 3): 1.40}


class _ShimPerfetto:
    """Minimal trace sink for TimelineSim (collective windows only)."""

    def __init__(self):
        self.collectives = []

    def add_event(self, process, track, name, start, dur, args=None):
        if track == "COLLECTIVE_CORES" and dur > 5000:
            self.collectives.append((start, start + dur))

    def __getattr__(self, name):
        return lambda *a, **k: None


def _sim_ag_windows(nc):
    """Run the cost-model timeline sim; return sorted collective windows."""
    import concourse.timeline_sim as ts
    old = ts._build_perfetto
    try:
        ts._build_perfetto = lambda core_id: _ShimPerfetto()
        sim = ts.TimelineSim(nc, trace=True)
        total = sim.simulate()
        return total, sorted(sim.perfetto.collectives)
    finally:
        ts._build_perfetto = old


def build_tuned(meta):
    """Multi-pass build: pilot sims fix the qv-production scheduler pins.

    The pin times come from the v2 timeline sim; the list scheduler runs the
    v1 cost model whose clock is slightly faster, hence the scale factor.
    """
    nc = build(dict(meta))
    try:
        total, ags = _sim_ag_windows(nc)
        nch = len(CHUNKS)
        best = (total, nc)
        for scale in (0.85, 1.0):
            a = ags
            for _ in range(2):
                if len(a) < 2 * nch:
                    break
                waits = {}
                for l in (1, 2):
                    for c in range(nch):
                        end = a[(l - 1) * nch + c][1]
                        waits[(l, c)] = (end * scale + 5e3) / 1e6
                m2 = dict(meta)
                m2["qv_waits"] = tuple(sorted(waits.items()))
                nc2 = build(m2)
                t2, a = _sim_ag_windows(nc2)
                if t2 < best[0]:
                    best = (t2, nc2)
        return best[1]
    except Exception:
        pass
    return nc


def wrap_idxs_block(idx):
    """Wrap one gather call's indices: idx j -> [j%16, j//16], tiled to 128 parts."""
    n = len(idx)
    assert n % 16 == 0
    w = np.asarray(idx, np.int16).reshape(n // 16, 16).T
    return np.tile(w, (8, 1))


def colmaj128(v):
    """Edge-scalar array -> [128, n/128] layout (edge j at [j%128, j//128])."""
    v = np.asarray(v)
    n = v.shape[0]
    assert n % 128 == 0
    return v.reshape(n // 128, 128).T.copy()


# ---------------------------------------------------------------------------
# host-side preprocessing
# ---------------------------------------------------------------------------

def prep(inputs):
    dtab = ml_dtypes.bfloat16
    x = np.asarray(inputs["x"], np.float32)
    edge_index = np.asarray(inputs["edge_index"])
    edge_attr = np.asarray(inputs["edge_attr"], np.float32)[:, 0]
    batch = np.asarray(inputs["batch"]).astype(np.int64)
    clinical = np.asarray(inputs["clinical"], np.float32)
    Wk, bk = np.asarray(inputs["Wk"], np.float32), np.asarray(inputs["bk"], np.float32)
    Wq, bq = np.asarray(inputs["Wq"], np.float32), np.asarray(inputs["bq"], np.float32)
    Wv, bv = np.asarray(inputs["Wv"], np.float32), np.asarray(inputs["bv"], np.float32)
    Ws, bs = np.asarray(inputs["Ws"], np.float32), np.asarray(inputs["bs"], np.float32)
    We, be = np.asarray(inputs["We"], np.float32), np.asarray(inputs["be"], np.float32)
    gamma = np.asarray(inputs["gamma"], np.float32)
    beta = np.asarray(inputs["beta"], np.float32)
    rmean = np.asarray(inputs["rmean"], np.float32)
    rvar = np.asarray(inputs["rvar"], np.float32)
    Wc, bc = np.asarray(inputs["Wc"], np.float32), np.asarray(inputs["bc"], np.float32)

    src = edge_index[0].astype(np.int64)
    dst = edge_index[1].astype(np.int64)

    # BN folded: A*x + B
    A = gamma / np.sqrt(rvar + EPS)
    B = beta - rmean * A
    bgate = bk + bq + be          # folded into k table
    # rank-blocked row id of the src node in the qv table
    rb_row = (src // NPR) * NPAD + (src % NPR)

    e_rank = dst // NPR
    epoch = (rb_row >= SPLIT).astype(np.int64)
    dst_local = dst - e_rank * NPR
    group = dst_local // 128
    dst_rel = dst_local % 128

    # per (rank, epoch, group) counts -> per-group caps (max over ranks)
    counts = {}
    for ep in (0, 1):
        cnt = np.zeros((R, NGRP), np.int64)
        m = epoch == ep
        np.add.at(cnt, (e_rank[m], group[m]), 1)
        counts[ep] = cnt
    T0g = np.maximum(1, np.ceil(counts[0].max(axis=0) / 128).astype(np.int64))
    T1g = np.maximum(1, np.ceil(counts[1].max(axis=0) / 128).astype(np.int64))
    Tg = T0g + T1g

    # graph counts for mean pooling
    cntg = np.bincount(batch, minlength=G).astype(np.float32)
    inv_cnt = 1.0 / np.maximum(cntg, 1.0)

    # sorted edge arrays: by rank, group, epoch, dst
    order = np.lexsort((dst, epoch, group, e_rank))
    src_s, dst_rel_s, attr_s = rb_row[order], dst_rel[order], edge_attr[order]
    ep_s, rank_s, grp_s = epoch[order], e_rank[order], group[order]
    key = ((rank_s * NGRP + grp_s) * 2 + ep_s)
    starts = np.searchsorted(key, np.arange(R * NGRP * 2 + 1))

    # meta pack layout per group (int16 cols): gidx0 | gidx1 | kidx | dst | attr
    MW = (18 * Tg).astype(np.int64)          # per-group meta width
    moff = np.zeros(NGRP + 1, np.int64)
    np.cumsum(MW, out=moff[1:])
    MTOT = int(moff[-1])

    iota_rep = np.tile(np.arange(128, dtype=np.float32), (128, 1))
    We_rep = np.stack([np.tile(We[l, 0], (128, 1)) for l in range(NLAYER)])
    Wqv = np.concatenate([Wq, Wv], axis=2)   # [L, H, 2H]
    bias_qv = np.zeros((NLAYER, 128, 2 * H), np.float32)
    bias_k = np.zeros((NLAYER, 128, H), np.float32)
    for l in range(NLAYER):
        bias_k[l, :, :] = bgate[l][None, :]
        bias_qv[l, :, H:2 * H] = bv[l][None, :]
    has_bias_qv = bool(np.any(bias_qv != 0))
    has_bias_k = bool(np.any(bias_k != 0))
    has_bs = bool(np.any(bs != 0))
    has_bc = bool(np.any(bc != 0))

    x_rb = np.zeros((R * 128, NPAD), np.float32)
    for r in range(R):
        lo, hi = r * NPR, min((r + 1) * NPR, N)
        x_rb[r * 128:(r + 1) * 128, 0:hi - lo] = x[lo:hi].T

    in_maps = []
    for r in range(R):
        meta_t = np.zeros((128, MTOT), np.int16)
        for g in range(NGRP):
            t0, t1, tg = int(T0g[g]), int(T1g[g]), int(Tg[g])
            o = moff[g]
            dstc = np.full((tg * 128,), -1.0, np.float32)
            attrc = np.zeros((tg * 128,), np.float32)
            kdx = np.zeros((tg * 128,), np.int64)
            for ep, cap, base in ((0, t0, 0), (1, t1, t0)):
                k = (r * NGRP + g) * 2 + ep
                s0 = starts[k]
                n = int(counts[ep][r, g])
                e0 = base * 128
                dstc[e0:e0 + n] = dst_rel_s[s0:s0 + n]
                attrc[e0:e0 + n] = attr_s[s0:s0 + n]
                idx = np.zeros((cap * 128,), np.int64)
                idx[:n] = src_s[s0:s0 + n] - ep * SPLIT
                go = o + (0 if ep == 0 else t0 * 8)
                meta_t[:, go:go + cap * 8] = wrap_idxs_block(idx)
                kdx[e0:e0 + n] = g * 128 + dst_rel_s[s0:s0 + n]
            meta_t[:, o + tg * 8:o + tg * 16] = wrap_idxs_block(kdx)
            meta_t[:, o + tg * 16:o + tg * 17] = (
                colmaj128(dstc).astype(dtab).view(np.int16))
            meta_t[:, o + tg * 17:o + tg * 18] = (
                colmaj128(attrc).astype(dtab).view(np.int16))
        # pooling indicator with 1/cnt folded
        IndT = np.zeros((NPAD, G), np.float32)
        lo, hi = r * NPR, min((r + 1) * NPR, N)
        IndT[np.arange(hi - lo), batch[lo:hi]] = inv_cnt[batch[lo:hi]]
        im = {
            "x_rb": x_rb.astype(ml_dtypes.float8_e4m3),
            "xT_loc": x_rb[r * 128:(r + 1) * 128].astype(dtab),
            "meta": meta_t,
            "Wk": Wk.astype(dtab), "Wqv": Wqv.astype(dtab), "Ws": Ws.astype(dtab),
            "We_rep": We_rep.astype(dtab),
            "bias_qv": bias_qv,
            "bias_k": bias_k,
            "bs_col": bs.reshape(NLAYER, H, 1),
            "A_col": A.reshape(NLAYER, H, 1),
            "B_col": B.reshape(NLAYER, H, 1),
            "iota_rep": iota_rep.astype(dtab),
            "IndT": IndT.astype(dtab),
            "clinT": clinical.T.copy(),
            "Wc_h": Wc[0:H], "Wc_c": Wc[H:H + NCLIN],
            "bc_rep": np.tile(bc, (G, 1)),
        }
        in_maps.append(im)
    meta = dict(T0g=tuple(int(t) for t in T0g), T1g=tuple(int(t) for t in T1g),
                has_bias_qv=has_bias_qv, has_bias_k=has_bias_k,
                has_bs=has_bs, has_bc=has_bc)
    return in_maps, meta


# ---------------------------------------------------------------------------
# device program
# ---------------------------------------------------------------------------

def build(meta):
    T0g, T1g = meta["T0g"], meta["T1g"]
    Tg = [a + b for a, b in zip(T0g, T1g)]
    moff = [0]
    for g in range(NGRP):
        moff.append(moff[-1] + 18 * Tg[g])
    MTOT = moff[-1]
    parts = meta.get("parts", 4)
    qv_waits = dict(meta.get("qv_waits", ())) or QV_WAITS
    DT = BF16

    # chunk column ranges
    chk = []
    g0 = 0
    for ng in CHUNKS:
        chk.append((g0, ng, g0 * 128, ng * 128))
        g0 += ng

    nc = bacc.Bacc("TRN2", target_bir_lowering=False, debug=False, num_devices=R)

    def din(name, shape, dt):
        return nc.dram_tensor(name, shape, dt, kind="ExternalInput").ap()

    t_x_rb = din("x_rb", [R * 128, NPAD], F8)
    t_xT_loc = din("xT_loc", [128, NPAD], DT)
    t_meta = din("meta", [128, MTOT], I16)
    t_Wk = din("Wk", [NLAYER, H, H], DT)
    t_Wqv = din("Wqv", [NLAYER, H, 2 * H], DT)
    t_Ws = din("Ws", [NLAYER, H, H], DT)
    t_We = din("We_rep", [NLAYER, 128, H], DT)
    t_bias_qv = din("bias_qv", [NLAYER, 128, 2 * H], F32)
    t_bias_k = din("bias_k", [NLAYER, 128, H], F32)
    t_bs = din("bs_col", [NLAYER, H, 1], F32)
    t_A = din("A_col", [NLAYER, H, 1], F32)
    t_B = din("B_col", [NLAYER, H, 1], F32)
    t_iota = din("iota_rep", [128, 128], DT)
    t_IndT = din("IndT", [NPAD, G], DT)
    t_clinT = din("clinT", [NCLIN, G], F32)
    t_Wc_h = din("Wc_h", [H, NCLS], F32)
    t_Wc_c = din("Wc_c", [NCLIN, NCLS], F32)
    t_bc = din("bc_rep", [G, NCLS], F32)

    t_out = nc.dram_tensor("out", [G, NCLS], F32, kind="ExternalOutput").ap()

    # double-buffered node tables (parity = layer % 2)
    qv_tab = [nc.dram_tensor(f"qv{p}", [NTOT, 2 * H], DT).ap() for p in range(2)]
    k_tab = [nc.dram_tensor(f"k{p}", [NPAD, H], DT).ap() for p in range(2)]
    # per (layer, chunk) collective buffers
    h_loc = [[nc.dram_tensor(f"hl{l}_{c}", [128, CHUNKS[c] * 128], F8).ap()
              for c in range(len(CHUNKS))] for l in range(2)]
    ag_out = [[nc.dram_tensor(f"ag{l}_{c}", [R * 128, CHUNKS[c] * 128], F8,
                              addr_space="Shared").ap()
               for c in range(len(CHUNKS))] for l in range(2)]
    pool_in = nc.dram_tensor("pool_in", [G, H], F32).ap()
    pool_out = nc.dram_tensor("pool_out", [R * G, H], F32,
                              addr_space="Shared").ap()

    with tile.TileContext(nc) as tc:
        import contextlib
        with contextlib.ExitStack() as ctx:
            consts = ctx.enter_context(tc.tile_pool(name="consts", bufs=1))
            hsb = ctx.enter_context(tc.tile_pool(name="hsb", bufs=1))
            lhp = ctx.enter_context(tc.tile_pool(name="lhp", bufs=2))
            stg = ctx.enter_context(tc.tile_pool(name="stg", bufs=3))
            edg = ctx.enter_context(tc.tile_pool(name="edg", bufs=3))
            edm = ctx.enter_context(tc.tile_pool(name="edm", bufs=2))
            pnode = ctx.enter_context(tc.tile_pool(name="pnode", bufs=3, space="PSUM"))
            pedge = ctx.enter_context(tc.tile_pool(name="pedge", bufs=2, space="PSUM"))
            ppool = ctx.enter_context(tc.tile_pool(name="ppool", bufs=1, space="PSUM"))

            _cid = [0]

            def load_const(src_ap, shape, dt):
                _cid[0] += 1
                t = consts.tile(shape, dt, tag=f"c{_cid[0]}_{src_ap.tensor.name}")
                nc.sync.dma_start(t[:], src_ap)
                return t

            Wk_t = [load_const(t_Wk[l], [H, H], DT) for l in range(NLAYER)]
            Wqv_t = [load_const(t_Wqv[l], [H, 2 * H], DT) for l in range(NLAYER)]
            Ws_t = [load_const(t_Ws[l], [H, H], DT) for l in range(NLAYER)]
            We_t = [load_const(t_We[l], [128, H], DT) for l in range(NLAYER)]
            bias_qv_t = [load_const(t_bias_qv[l], [128, 2 * H], F32)
                         for l in range(NLAYER)] if meta["has_bias_qv"] else None
            bias_k_t = [load_const(t_bias_k[l], [128, H], F32)
                        for l in range(NLAYER)] if meta["has_bias_k"] else None
            bs_t = [load_const(t_bs[l], [H, 1], F32) for l in range(NLAYER)]
            A_t = [load_const(t_A[l], [H, 1], F32) for l in range(NLAYER)]
            B_t = [load_const(t_B[l], [H, 1], F32) for l in range(NLAYER)]
            iota_t = load_const(t_iota, [128, 128], DT)
            ident = consts.tile([128, 128], DT)
            make_identity(nc, ident[:])
            identf = consts.tile([128, 128], F32)
            make_identity(nc, identf[:])
            clin_t = load_const(t_clinT, [NCLIN, G], F32)
            Wch_t = load_const(t_Wc_h, [H, NCLS], F32)
            Wcc_t = load_const(t_Wc_c, [NCLIN, NCLS], F32)
            bc_t = load_const(t_bc, [G, NCLS], F32) if meta["has_bc"] else None

            # double-buffered feature-major accumulators / h tiles
            hs_t = [hsb.tile([128, NPAD], F32, tag=f"hs{p}", name=f"hs{p}")
                    for p in range(2)]
            h3_t = [hsb.tile([128, NPAD], DT, tag=f"h3{p}", name=f"h3{p}")
                    for p in range(2)]

            # ---------------- node-table production helpers ----------------
            def node_sk(l, c0, csz, rhs_src):
                """s-table cols [c0,c0+csz) into hs_t[l%2] + k rows into k_tab.

                rhs_src: feature-major h source; either an SBUF AP (h3 of the
                previous layer) or None (layer 0 -> stream xT_loc).
                """
                hs = hs_t[l % 2]
                for q0 in range(0, csz, 512):
                    qs = min(512, csz - q0)
                    if rhs_src is None:
                        lh = lhp.tile([128, qs], DT, tag="lhx")
                        nc.sync.dma_start(lh[:], t_xT_loc[:, c0 + q0:c0 + q0 + qs])
                        rhs = lh[:]
                    else:
                        rhs = rhs_src[:, c0 + q0:c0 + q0 + qs]
                    ps = pnode.tile([128, 512], F32, tag="pn")
                    nc.tensor.matmul(out=ps[:, 0:qs], lhsT=Ws_t[l][:], rhs=rhs,
                                     start=True, stop=True)
                    if meta["has_bs"]:
                        nc.scalar.activation(hs[:, c0 + q0:c0 + q0 + qs],
                                             ps[:, 0:qs], AF.Identity,
                                             bias=bs_t[l][:], scale=1.0)
                    else:
                        nc.scalar.activation(hs[:, c0 + q0:c0 + q0 + qs],
                                             ps[:, 0:qs], AF.Copy)
                    # k rows for these cols (4 blocks -> one 512-row write)
                    nblk = qs // 128
                    psk = pnode.tile([128, 512], F32, tag="pn")
                    for s in range(nblk):
                        nc.tensor.matmul(
                            out=psk[:, s * 128:(s + 1) * 128],
                            lhsT=rhs[:, s * 128:(s + 1) * 128] if rhs_src is None
                            else rhs_src[:, c0 + q0 + s * 128:c0 + q0 + (s + 1) * 128],
                            rhs=Wk_t[l][:], start=True, stop=True)
                    stk = stg.tile([128, 512], DT, tag="stk")
                    if meta["has_bias_k"]:
                        for s in range(nblk):
                            nc.vector.tensor_tensor(
                                out=stk[:, s * 128:(s + 1) * 128],
                                in0=psk[:, s * 128:(s + 1) * 128],
                                in1=bias_k_t[l][:], op=OP.add)
                    else:
                        nc.scalar.activation(stk[:, 0:nblk * 128],
                                             psk[:, 0:nblk * 128], AF.Copy)
                    dst_ap = k_tab[l % 2][c0 + q0:c0 + q0 + qs, :]
                    nc.sync.dma_start(
                        dst_ap.rearrange("(b p) h -> p b h", p=128),
                        stk[:, 0:nblk * 128])

            def cp(dst_ap, src_ap, dve):
                if dve:
                    nc.vector.tensor_copy(dst_ap, src_ap)
                else:
                    nc.scalar.activation(dst_ap, src_ap, AF.Copy)

            def node_qv(l, ci, split_eng=False):
                """qv rows for all 8 rank blocks of chunk ci into qv_tab[l%2]."""
                gc0, gng, c0, csz = chk[ci]
                for rb in range(R):
                    # layer 0 runs standalone (DVE idle); interleaved layers
                    # keep DVE clear for edge-phase gate math
                    dve = (rb % 2 == 0) if l == 0 else (split_eng and rb % 4 < 3)
                    if l == 0:
                        src_ap = t_x_rb[rb * 128:(rb + 1) * 128, c0:c0 + csz]
                    else:
                        src_ap = ag_out[(l - 1) % 2][ci][
                            rb * 128:(rb + 1) * 128, :]
                    lh8 = lhp.tile([128, csz], F8, tag="lh8")
                    nc.sync.dma_start(lh8[:], src_ap)
                    lh = lhp.tile([128, csz], DT, tag="lh")
                    cp(lh[:], lh8[:], dve)
                    nblk = csz // 128
                    for s0 in range(0, nblk, 4):
                        nb = min(4, nblk - s0)
                        st = stg.tile([128, 4 * 256], DT, tag="st")
                        for sp in range(0, nb, 2):
                            np2 = min(2, nb - sp)
                            ps = pnode.tile([128, 512], F32, tag="pn")
                            for j in range(np2):
                                s = s0 + sp + j
                                nc.tensor.matmul(
                                    out=ps[:, j * 256:(j + 1) * 256],
                                    lhsT=lh[:, s * 128:(s + 1) * 128],
                                    rhs=Wqv_t[l][:], start=True, stop=True)
                            if meta["has_bias_qv"]:
                                for j in range(np2):
                                    nc.vector.tensor_tensor(
                                        out=st[:, (sp + j) * 256:(sp + j + 1) * 256],
                                        in0=ps[:, j * 256:(j + 1) * 256],
                                        in1=bias_qv_t[l][:], op=OP.add)
                            else:
                                cp(st[:, sp * 256:(sp + np2) * 256],
                                   ps[:, 0:np2 * 256], dve)
                        row = rb * NPAD + c0 + s0 * 128
                        dst_ap = qv_tab[l % 2][row:row + nb * 128, :]
                        nc.sync.dma_start(
                            dst_ap.rearrange("(b p) h -> p b h", p=128),
                            st[:, 0:nb * 256])

            # ---------------- edge phase helper ----------------
            def edge_group(l, g):
                hs = hs_t[l % 2]
                t0, t1, tg = T0g[g], T1g[g], Tg[g]
                o = moff[g]
                mt = edg.tile([128, 18 * tg], I16, tag="meta")
                nc.sync.dma_start(mt[:], t_meta[:, o:o + 18 * tg])
                gt = edg.tile([128, tg, 2 * H], DT, tag="gt")
                nc.gpsimd.dma_gather(
                    gt[:, 0:t0, :], qv_tab[l % 2][0:SPLIT, :],
                    mt[:, 0:t0 * 8], t0 * 128, t0 * 128, 2 * H,
                    single_packet=False)
                nc.gpsimd.dma_gather(
                    gt[:, t0:tg, :], qv_tab[l % 2][SPLIT:NTOT, :],
                    mt[:, t0 * 8:tg * 8], t1 * 128, t1 * 128, 2 * H,
                    single_packet=False)
                kt = edg.tile([128, tg, H], DT, tag="kt")
                nc.gpsimd.dma_gather(
                    kt[:], k_tab[l % 2][:],
                    mt[:, tg * 8:tg * 16], tg * 128, tg * 128, H,
                    single_packet=False)
                dsl = mt[:, tg * 16:tg * 17].bitcast(DT)
                asl = mt[:, tg * 17:tg * 18].bitcast(DT)
                S = edm.tile([128, tg, 128], DT, tag="S")
                if g % 2 == 0:
                    # 1x broadcast is_equal directly on DVE
                    nc.vector.tensor_tensor(
                        out=S[:],
                        in0=dsl.unsqueeze(2).to_broadcast([128, tg, 128]),
                        in1=iota_t[:].unsqueeze(1).to_broadcast([128, tg, 128]),
                        op=OP.is_equal)
                else:
                    # materialize dst replicated on ACT, then packed 2x compare
                    nc.scalar.activation(
                        S[:], dsl.unsqueeze(2).to_broadcast([128, tg, 128]),
                        AF.Copy)
                    nc.vector.tensor_tensor(
                        out=S[:], in0=S[:],
                        in1=iota_t[:].unsqueeze(1).to_broadcast([128, tg, 128]),
                        op=OP.is_equal)
                nc.vector.tensor_tensor(out=kt[:], in0=kt[:],
                                        in1=gt[:, :, 0:H], op=OP.add)
                # materialize attr replicated along H on ACT so the multiply
                # below runs packed (DVE 2x) instead of broadcast (1x)
                et = edm.tile([128, tg, H], DT, tag="et")
                nc.scalar.activation(
                    et[:], asl.unsqueeze(2).to_broadcast([128, tg, H]), AF.Copy)
                nc.vector.tensor_tensor(
                    out=et[:], in0=et[:],
                    in1=We_t[l][:].unsqueeze(1).to_broadcast([128, tg, H]),
                    op=OP.mult)
                nc.vector.tensor_tensor(out=kt[:], in0=kt[:], in1=et[:],
                                        op=OP.add)
                nc.scalar.activation(kt[:], kt[:], AF.Sigmoid)
                nc.vector.tensor_tensor(out=et[:], in0=kt[:],
                                        in1=gt[:, :, H:2 * H], op=OP.mult)
                pa = pedge.tile([128, 128], F32, tag="pa")
                for t in range(tg):
                    nc.tensor.matmul(out=pa[:], lhsT=et[:, t, :],
                                     rhs=S[:, t, :], start=(t == 0),
                                     stop=(t == tg - 1))
                nc.vector.tensor_tensor(
                    out=hs[:, g * 128:(g + 1) * 128],
                    in0=hs[:, g * 128:(g + 1) * 128], in1=pa[:], op=OP.add)

            def bn_chunk(l, ci):
                """leaky+BN for chunk ci cols -> h3_t[l%2]; returns col range."""
                _, _, c0, csz = chk[ci]
                hs, h3 = hs_t[l % 2], h3_t[l % 2]
                nc.scalar.activation(h3[:, c0:c0 + csz], hs[:, c0:c0 + csz],
                                     AF.Lrelu, alpha=SLOPE)
                nc.scalar.activation(h3[:, c0:c0 + csz], h3[:, c0:c0 + csz],
                                     AF.Identity, bias=B_t[l][:],
                                     scale=A_t[l][:])
                return c0, csz

            # ---------------- program ----------------
            # layer 0 node phase (from x)
            for (gc0, gng, c0, csz) in chk:
                node_sk(0, c0, csz, None)
            for ci in range(len(chk)):
                node_qv(0, ci)

            # pooling accumulator, fed per chunk during the last layer
            pp = ppool.tile([G, H], F32)

            def pool_chunk(ci):
                gc0, gng, c0, csz = chk[ci]
                h3f = h3_t[(NLAYER - 1) % 2]
                for c in range(gc0, gc0 + gng):
                    trp = ppool.tile([128, 128], DT, tag="ptr", bufs=2)
                    nc.tensor.transpose(out=trp[:],
                                        in_=h3f[:, c * 128:(c + 1) * 128],
                                        identity=ident[:])
                    hnode = stg.tile([128, 128], DT, tag="hnode")
                    cp(hnode[:], trp[:], c % 2 == 0)
                    ind_t = stg.tile([128, G], DT, tag="ind")
                    nc.sync.dma_start(ind_t[:], t_IndT[c * 128:(c + 1) * 128, :])
                    nc.tensor.matmul(out=pp[:], lhsT=ind_t[:], rhs=hnode[:],
                                     start=(c == 0), stop=(c == NGRP - 1))

            for l in range(NLAYER):
                for ci, (gc0, gng, c0, csz) in enumerate(chk):
                    if parts >= 2:
                        for g in range(gc0, gc0 + gng):
                            edge_group(l, g)
                    if parts < 3:
                        continue
                    c0_, csz_ = bn_chunk(l, ci)
                    if l < NLAYER - 1:
                        h3 = h3_t[l % 2]
                        h8 = stg.tile([128, csz], F8, tag="h8")
                        nc.scalar.activation(h8[:], h3[:, c0:c0 + csz], AF.Copy)
                        nc.sync.dma_start(h_loc[l][ci][:], h8[:])
                        nc.gpsimd.collective_compute(
                            "AllGather", OP.bypass,
                            replica_groups=[list(range(R))],
                            ins=[h_loc[l][ci][:]], outs=[ag_out[l % 2][ci][:]])
                        node_sk(l + 1, c0, csz, h3[:])
                        if ci >= 2:
                            with tc.tile_wait_until(qv_waits[(l + 1, ci - 2)]):
                                node_qv(l + 1, ci - 2)
                    elif parts >= 4:
                        pool_chunk(ci)
                if parts >= 3 and l < NLAYER - 1:
                    for cl in (len(chk) - 2, len(chk) - 1):
                        with tc.tile_wait_until(qv_waits[(l + 1, cl)]):
                            node_qv(l + 1, cl, split_eng=True)

            if parts < 4:
                z_dbg = stg.tile([G, NCLS], F32, tag="zsb")
                nc.vector.tensor_copy(z_dbg[:], hs_t[0][0:G, 0:NCLS])
                nc.sync.dma_start(t_out[:], z_dbg[:])
            else:
                # ---- pooling tail (pp accumulated per chunk above)
                pool_sb = stg.tile([G, H], F32, tag="poolsb")
                nc.vector.tensor_copy(pool_sb[:], pp[:])
                nc.sync.dma_start(pool_in[:], pool_sb[:])
                nc.gpsimd.collective_compute(
                    "AllGather", OP.bypass, replica_groups=[list(range(R))],
                    ins=[pool_in[:]], outs=[pool_out[:]])
                # sum the 8 partial pools
                pr = stg.tile([G, R, H], F32, tag="pr")
                nc.sync.dma_start(pr[:], pool_out[:].rearrange("(r g) h -> g r h", r=R))
                pooled = stg.tile([G, H], F32, tag="pooled")
                nc.vector.tensor_tensor(out=pooled[:], in0=pr[:, 0, :], in1=pr[:, 1, :],
                                        op=OP.add)
                for r in range(2, R):
                    nc.vector.tensor_tensor(out=pooled[:], in0=pooled[:],
                                            in1=pr[:, r, :], op=OP.add)
                # transpose pooled [G,H] -> [H,G]
                ptp = pedge.tile([H, G], F32, tag="pa")
                nc.tensor.transpose(out=ptp[:], in_=pooled[:], identity=identf[0:G, 0:G])
                pooledT = stg.tile([H, G], F32, tag="pooledT")
                nc.vector.tensor_copy(pooledT[:], ptp[:])
                zp = pedge.tile([G, NCLS], F32, tag="pa")
                nc.tensor.matmul(out=zp[:], lhsT=pooledT[:], rhs=Wch_t[:],
                                 start=True, stop=False)
                nc.tensor.matmul(out=zp[:], lhsT=clin_t[:], rhs=Wcc_t[:],
                                 start=False, stop=True)
                z_sb = stg.tile([G, NCLS], F32, tag="zsb")
                if meta["has_bc"]:
                    nc.vector.tensor_tensor(out=z_sb[:], in0=zp[:], in1=bc_t[:],
                                            op=OP.add)
                else:
                    nc.vector.tensor_copy(z_sb[:], zp[:])
                nc.sync.dma_start(t_out[:], z_sb[:])

    nc.compile()
    return nc


# ---------------------------------------------------------------------------

_CACHE = {}


def kernel(**inputs):
    in_maps, meta = prep(inputs)
    key = tuple(sorted((k, v) for k, v in meta.items()))
    if key not in _CACHE:
        _CACHE[key] = build_tuned(meta)
    nc = _CACHE[key]
    res = run_bass_kernel_spmd(nc, in_maps, list(range(R)))
    return np.asarray(res.results[0]["out"], np.float32)


def kernel_profiled(**inputs):
    """Like kernel() but also returns (exec_time_ns, trace_path)."""
    in_maps, meta = prep(inputs)
    key = tuple(sorted((k, v) for k, v in meta.items()))
    if key not in _CACHE:
        _CACHE[key] = build_tuned(meta)
    nc = _CACHE[key]
    res = run_bass_kernel_spmd(nc, in_maps, list(range(R)), trace=True)
    out = np.asarray(res.results[0]["out"], np.float32)
    trace_path = None
    if res.instructions_and_trace is not None:
        trace_path = res.instructions_and_trace[1]
    return out, res.exec_time_ns, trace_path


if __name__ == "__main__":
    pass
